# revision 18
# baseline (speedup 1.0000x reference)
"""AudioCrossAttention on 8 Trainium2 NeuronCores.

Sharding: data-parallel over batch (B=2) x tensor-parallel over heads
(16 heads -> 4 heads / 256 dims per core).  Core c handles batch c//4 and
head-group c%4.  Each core computes its 4 heads' attention plus the partial
output projection over its 256-dim slice; partials are summed on the host
(the unshard step) and bo (+ Wo@bv, since attn rows sum to 1) added there.

Everything on device flows in transposed layout ([d, s] / [skv, sq]) so no
transposes are ever needed.  All data is fp16 (fp8 anywhere in the q/k/v
path measurably breaks the 2e-2 error budget on this problem).

Performance structure (vs the naive version):
  * scores  scoresT[skv,sq] = kT_h.T @ qT_h  per head, K=64: the two heads
    of a pair live at base partitions 0/64 -> disjoint PE row groups -> the
    two matmuls execute CONCURRENTLY in the 128x128 array (2x throughput).
  * AV      [outT;den] = [v_h|1].T @ expT is split into two K=64 halves on
    row groups 0-1 / 2-3 (skv positions 0-63 / 64-127 of each tile), again
    concurrent; the two PSUM halves are summed during normalization.
  * exp     is the 2nd bottleneck (16.8M elems/core, 1 elem/lane/cycle on
    ACT).  ~1/4 of the score tiles are evicted by the DVE instead, using a
    Schraudolph-style exp: qT/kT carry a host-side x13.589 scale so the
    scores PSUM is directly in f16-exponent-bit units; one tensor_scalar_add
    (+15316, int16 out, bitcast f16) IS the approximate exp.  ACT tiles use
    the exact spline exp with the matching 1/184.66 scale.
  * the out-projection PSUM evictions are split between DVE and ACT; phase 2
    interleaves AV-pair matmuls of the previous head-pair between score
    matmuls of the current pair so the PE never head-blocks on the
    eviction-gated score PSUM pool.
"""

import sys

if '/opt/trn_rl_repo' not in sys.path:
    sys.path.insert(0, '/opt/trn_rl_repo')

import math
import numpy as np

B = 2
SQ = 2048
SKV = 2048
DIM = 1024
NUM_HEADS = 16
HEAD_DIM = 64
N_CORES = 8
HPC = 4          # heads per core
DSL = 256        # d_out slice per core
CH = 512         # sq chunk width
NCH = SQ // CH   # 4
KT = DIM // 128  # 8  d_in k-tiles
ST = SKV // 128  # 16 skv tiles

# qT/kT host prescale so scores psum == f16 exponent-bit units (x1024/ln2
# at the 1/8 softmax scale): bits = psum + B16 is the Schraudolph f16 exp.
A_FULL = (1024.0 / math.log(2.0)) * (HEAD_DIM ** -0.5)   # 184.664
QS = math.sqrt(A_FULL)                                    # 13.5891
SCALE_ACT = (HEAD_DIM ** -0.5) / A_FULL                   # exact-exp scale
B16 = 15360.0 - 44.0                                      # f16 bias - corr

_CACHE = {}


def _dve_tile(s2, j=0, c=0):
    """Which skv-tile evictions run on DVE instead of ACT (load balance)."""
    return s2 % 4 == 2 or s2 == 9 or (s2 == 5 and (j + c) % 2 == 1)


def _build():
    import concourse.bacc as bacc
    import concourse.mybir as mybir
    from concourse import tile

    F32 = mybir.dt.float32
    F16 = mybir.dt.float16
    I16 = mybir.dt.int16
    AF = mybir.ActivationFunctionType
    ALU = mybir.AluOpType

    nc = bacc.Bacc("TRN2", target_bir_lowering=False, debug=False,
                   num_devices=N_CORES)

    xq = nc.dram_tensor("xq", [DIM, SQ], F16, kind="ExternalInput")
    xa = nc.dram_tensor("xa", [DIM, SKV], F16, kind="ExternalInput")
    wq = nc.dram_tensor("wq", [DIM, DSL], F16, kind="ExternalInput")
    wk = nc.dram_tensor("wk", [DIM, DSL], F16, kind="ExternalInput")
    wv = nc.dram_tensor("wv", [DIM, DSL], F16, kind="ExternalInput")
    wo = nc.dram_tensor("wo", [DSL, DIM], F16, kind="ExternalInput")
    emb2 = nc.dram_tensor("emb2", [128, SKV], F16, kind="ExternalInput")
    bq2 = nc.dram_tensor("bq2", [128, 2], F32, kind="ExternalInput")
    bk2 = nc.dram_tensor("bk2", [128, 2], F32, kind="ExternalInput")
    out = nc.dram_tensor("out", [DIM, SQ], F16, kind="ExternalOutput")

    with tile.TileContext(nc) as tc:
        with tc.tile_pool(name="consts", bufs=1) as consts, \
             tc.tile_pool(name="big", bufs=1) as big, \
             tc.tile_pool(name="xqp", bufs=2) as xqp, \
             tc.tile_pool(name="xap", bufs=2) as xap, \
             tc.tile_pool(name="etp", bufs=3) as etp, \
             tc.tile_pool(name="evp", bufs=4) as evp, \
             tc.tile_pool(name="smallp", bufs=2) as smallp, \
             tc.tile_pool(name="ps512", bufs=2, space="PSUM") as ps512, \
             tc.tile_pool(name="ps1024", bufs=2, space="PSUM") as ps1024, \
             tc.tile_pool(name="psav", bufs=2, space="PSUM") as psav:

            # ---- constants (xa chunk 0 + wk first: k-proj is the first
            # consumer, so the front of the DMA queue feeds it) ----
            xat0 = xap.tile([128, KT, CH], F16, tag="xa", name="xa0")
            nc.sync.dma_start(
                out=xat0, in_=xa[:, 0:CH].rearrange("(kt p) m -> p kt m", p=128))
            wk_sb = consts.tile([128, KT, DSL], F16, tag="wk")
            nc.sync.dma_start(out=wk_sb, in_=wk.rearrange("(kt p) m -> p kt m", p=128))
            bk_sb = consts.tile([128, 2], F32, tag="bk")
            nc.sync.dma_start(out=bk_sb, in_=bk2[:, :])
            emb_sb = consts.tile([128, SKV], F16, tag="emb")
            nc.sync.dma_start(out=emb_sb, in_=emb2[:, :])
            wv_sb = consts.tile([128, KT, DSL], F16, tag="wv")
            nc.sync.dma_start(out=wv_sb, in_=wv.rearrange("(kt p) m -> p kt m", p=128))
            wq_sb = consts.tile([128, KT, DSL], F16, tag="wq")
            nc.sync.dma_start(out=wq_sb, in_=wq.rearrange("(kt p) m -> p kt m", p=128))
            bq_sb = consts.tile([128, 2], F32, tag="bq")
            nc.sync.dma_start(out=bq_sb, in_=bq2[:, :])
            wo_sb = consts.tile([128, 2, DIM], F16, tag="wo")
            nc.sync.dma_start(out=wo_sb, in_=wo.rearrange("(kt p) m -> p kt m", p=128))

            # ---- persistent activations ----
            qT = big.tile([128, 2, SQ], F16, tag="qT")
            kT = big.tile([128, 2, SKV], F16, tag="kT")
            oT0 = big.tile([128, SQ], F16, tag="oT0")
            oT1 = big.tile([128, SQ], F16, tag="oT1")
            oTs = [oT0, oT1]
            v4 = big.tile([128, ST, HPC, 68], F16, tag="v4")
            nc.vector.memset(v4[:, :, :, 64:65], 1.0)

            ets = {}
            next_s2 = {}

            def _scores(j, c, s2):
                """One skv-tile of scores for head pair (2j, 2j+1): two
                concurrent K=64 matmuls (rows 0-63 / 64-127), then the exp
                eviction to f16 on ACT (exact) or DVE (Schraudolph)."""
                et = ets[(j, c)]
                pss = ps1024.tile([128, 2, CH], F32, tag="sc",
                                  name=f"pss{j}_{c}_{s2}")
                for half in range(2):
                    pb = half * 64
                    nc.tensor.matmul(
                        pss[:, half, :],
                        kT[pb:pb + 64, j, s2 * 128:(s2 + 1) * 128],
                        qT[pb:pb + 64, j, c * CH:(c + 1) * CH],
                        start=True, stop=True)
                if _dve_tile(s2, j, c):
                    nc.vector.tensor_scalar_add(et[:, s2].bitcast(I16), pss,
                                                B16)
                else:
                    nc.scalar.activation(et[:, s2], pss, AF.Exp,
                                         scale=SCALE_ACT)
                next_s2[(j, c)] = s2 + 1

            def _qproj(c, xqt):
                psq = [ps512.tile([128, CH], F32, tag="mm", name=f"psq{c}_{i}")
                       for i in range(2)]
                for kt in range(KT):
                    for mt in range(2):
                        nc.tensor.matmul(psq[mt],
                                         wq_sb[:, kt, mt * 128:(mt + 1) * 128],
                                         xqt[:, kt, :],
                                         start=(kt == 0), stop=(kt == KT - 1))
                for mt in range(2):
                    nc.vector.tensor_scalar_add(qT[:, mt, c * CH:(c + 1) * CH],
                                                psq[mt], bq_sb[:, mt:mt + 1])

            # ---- phase 1: k/v/q projections with prescheduled score
            # tiles sprinkled between the projection sub-blocks so the
            # ACT/DVE exp stream is saturated from the start.  Only the
            # first three head-pairs fit in the et pool (3 bufs). ----
            PRESCHED = [(0, 0), (1, 0), (0, 1)]   # (j, c) in phase-2 order
            qdone = [False] * NCH

            def _sprinkle(n, cur_c):
                issued = 0
                for (pj, pc) in PRESCHED:
                    if pc > cur_c or not qdone[pc]:
                        continue
                    if (pj, pc) not in ets:
                        ets[(pj, pc)] = etp.tile([128, ST, 2, CH], F16,
                                                 tag="et", name=f"et{pj}_{pc}")
                    lim = min(4 * (cur_c + 1), ST if (pj, pc) != PRESCHED[-1] else 8)
                    while next_s2.get((pj, pc), 0) < lim and issued < n:
                        _scores(pj, pc, next_s2.get((pj, pc), 0))
                        issued += 1
                    if issued >= n:
                        break

            for c in range(NCH):
                csl = slice(c * CH, (c + 1) * CH)
                if c == 0:
                    xat = xat0
                else:
                    xat = xap.tile([128, KT, CH], F16, tag="xa", name=f"xa{c}")
                    nc.sync.dma_start(
                        out=xat, in_=xa[:, csl].rearrange("(kt p) m -> p kt m", p=128))
                psk = [ps512.tile([128, CH], F32, tag="mm", name=f"psk{c}_{i}")
                       for i in range(2)]
                for kt in range(KT):
                    for mt in range(2):
                        nc.tensor.matmul(psk[mt],
                                         wk_sb[:, kt, mt * 128:(mt + 1) * 128],
                                         xat[:, kt, :],
                                         start=(kt == 0), stop=(kt == KT - 1))
                for mt in range(2):
                    # kT = (psum + bk) + emb (emb rows duplicated per head half)
                    nc.vector.scalar_tensor_tensor(
                        kT[:, mt, csl], psk[mt], bk_sb[:, mt:mt + 1],
                        emb_sb[:, csl], ALU.add, ALU.add)
                _sprinkle(2, c)
                for jj in range(HPC):
                    st = c * HPC + jj
                    psv = psav.tile([128, CH], F32, tag="av", name=f"psv{st}")
                    for kt in range(KT):
                        nc.tensor.matmul(psv[:, 0:DSL],
                                         xat[:, kt, jj * 128:(jj + 1) * 128],
                                         wv_sb[:, kt, :],
                                         start=(kt == 0), stop=(kt == KT - 1))
                    nc.vector.tensor_copy(
                        v4[:, st, :, 0:64],
                        psv[:, 0:DSL].rearrange("p (g m) -> p g m", g=HPC))
                    _sprinkle(2, c)
                xqt = xqp.tile([128, KT, CH], F16, tag="xq", name=f"xq{c}")
                nc.sync.dma_start(
                    out=xqt, in_=xq[:, csl].rearrange("(kt p) m -> p kt m", p=128))
                _qproj(c, xqt)
                qdone[c] = True
                _sprinkle(6 if c < NCH - 1 else 64, c)

            # ---- phase 2: software-pipelined pairs: AV (row-split
            # concurrent matmul pairs) of the previous head-pair interleaved
            # between score tiles of the current pair ----
            def _av_mm(j, c, h, s2, pstate):
                et = ets[(j, c)]
                if s2 == 0:
                    pstate[h] = (
                        psav.tile([128, CH], F32, tag="av", name=f"pavA{h}_{c}"),
                        psav.tile([128, CH], F32, tag="av", name=f"pavB{h}_{c}"),
                    )
                psA, psB = pstate[h]
                for half in range(2):
                    pb = half * 64
                    nc.tensor.matmul((psA if half == 0 else psB)[0:65, :],
                                     v4[pb:pb + 64, s2, h, 0:65],
                                     et[pb:pb + 64, s2, h % 2],
                                     start=(s2 == 0), stop=(s2 == ST - 1))

            def _normalize(j, c, h, pstate):
                psA, psB = pstate[h]
                pb = (h % 2) * 64
                sbB = smallp.tile([65, CH], F32, tag="sbB", name=f"sbB{h}_{c}")
                if h % 2 == 0:
                    nc.vector.tensor_copy(sbB, psB[0:65, :])
                else:
                    nc.scalar.copy(sbB, psB[0:65, :])
                t = smallp.tile([65, CH], F32, tag="t", name=f"t{h}_{c}")
                nc.vector.tensor_tensor(t, psA[0:65, :], sbB,
                                        mybir.AluOpType.add)
                # the custom-DVE reciprocal cannot remap partitions (in/out
                # bases must match); standard tensor_copy can, so stage the
                # denominator row down to partition 0 first
                denrow = smallp.tile([1, CH], F32, tag="den")
                nc.vector.tensor_copy(denrow, t[64:65, :])
                drec = smallp.tile([1, CH], F32, tag="drec")
                nc.vector.reciprocal_approx_fast(drec, denrow)
                bc_sb = smallp.tile([64, CH], F32, tag="bcs")
                nc.gpsimd.partition_broadcast(bc_sb, drec)
                nc.vector.tensor_mul(oTs[j][pb:pb + 64, c * CH:(c + 1) * CH],
                                     t[0:64, :], bc_sb)

            def _outproj_e(c, e):
                pso = ps512.tile([128, CH], F32, tag="mm", name=f"pso{c}_{e}")
                for kt in range(2):
                    nc.tensor.matmul(pso, wo_sb[:, kt, e * 128:(e + 1) * 128],
                                     oTs[kt][:, c * CH:(c + 1) * CH],
                                     start=(kt == 0), stop=(kt == 1))
                ot_sb = evp.tile([128, CH], F16, tag="ev", name=f"ot{c}_{e}")
                if e % 2 == 0:
                    nc.vector.tensor_copy(ot_sb, pso)
                else:
                    nc.scalar.copy(ot_sb, pso)
                nc.sync.dma_start(
                    out=out[e * 128:(e + 1) * 128, c * CH:(c + 1) * CH],
                    in_=ot_sb)

            def _outproj(c):
                for e in range(8):
                    _outproj_e(c, e)

            pairs = [(c, j) for c in range(NCH) for j in range(2)]
            for i, (c, j) in enumerate(pairs):
                if (j, c) not in ets:
                    ets[(j, c)] = etp.tile([128, ST, 2, CH], F16,
                                           tag="et", name=f"et{j}_{c}")
                prev = pairs[i - 1] if i > 0 else None
                avq = []
                pstate = {}
                if prev is not None:
                    pc_, pj_ = prev
                    avq = [(pj_, pc_, 2 * pj_ + hh, s2)
                           for hh in range(2) for s2 in range(ST)]
                sq = list(range(next_s2.get((j, c), 0), ST))
                if i == len(pairs) - 1:
                    avq += [(j, c, 2 * j + hh, s2)
                            for hh in range(2) for s2 in range(ST)]
                ai = 0
                while ai < len(avq) or sq:
                    for _ in range(2):
                        if ai < len(avq):
                            aj, ac, ah, as2 = avq[ai]
                            _av_mm(aj, ac, ah, as2, pstate)
                            if as2 == ST - 1:
                                _normalize(aj, ac, ah, pstate)
                            ai += 1
                    if sq:
                        _scores(j, c, sq.pop(0))
                if j == 1 and c > 0:
                    _outproj(c - 1)
            _outproj(NCH - 1)

    nc.compile()
    return nc


def _make_runner(nc):
    """Build a reusable jitted SPMD executor (mirrors bass2jax.run_bass_via_pjrt)."""
    import jax
    import numpy as _np
    from jax.sharding import Mesh, PartitionSpec
    from jax.experimental.shard_map import shard_map
    import concourse.mybir as mybir
    from concourse.bass2jax import (_bass_exec_p, install_neuronx_cc_hook,
                                    partition_id_tensor)

    install_neuronx_cc_hook()
    partition_name = nc.partition_id_tensor.name if nc.partition_id_tensor else None

    in_names, out_names, out_avals, zero_outs = [], [], [], []
    for alloc in nc.m.functions[0].allocations:
        if not isinstance(alloc, mybir.MemoryLocationSet):
            continue
        name = alloc.memorylocations[0].name
        if alloc.kind == "ExternalInput":
            if name != partition_name:
                in_names.append(name)
        elif alloc.kind == "ExternalOutput":
            shape = tuple(alloc.tensor_shape)
            dtype = mybir.dt.np(alloc.dtype)
            out_names.append(name)
            out_avals.append(jax.core.ShapedArray(shape, dtype))
            zero_outs.append(_np.zeros(shape, dtype))
    n_params = len(in_names)
    n_outs = len(out_avals)
    all_in_names = list(in_names) + list(out_names)
    if partition_name is not None:
        all_in_names.append(partition_name)
    donate = tuple(range(n_params, n_params + n_outs))

    def _body(*args):
        operands = list(args)
        if partition_name is not None:
            operands.append(partition_id_tensor())
        outs = _bass_exec_p.bind(
            *operands,
            out_avals=tuple(out_avals),
            in_names=tuple(all_in_names),
            out_names=tuple(out_names),
            lowering_input_output_aliases=(),
            sim_require_finite=True,
            sim_require_nnan=True,
            nc=nc,
        )
        return tuple(outs)

    devices = jax.devices()[:N_CORES]
    mesh = Mesh(np.asarray(devices), ("core",))
    in_specs = (PartitionSpec("core"),) * (n_params + n_outs)
    out_specs = (PartitionSpec("core"),) * n_outs
    sharded = jax.jit(
        shard_map(_body, mesh=mesh, in_specs=in_specs, out_specs=out_specs,
                  check_rep=False),
        donate_argnums=donate, keep_unused=True)
    # non-donating variant for repeat-timing with device-resident operands
    sharded_nd = jax.jit(
        shard_map(_body, mesh=mesh, in_specs=in_specs, out_specs=out_specs,
                  check_rep=False),
        keep_unused=True)

    def _concat(in_maps):
        concat_in = [
            np.concatenate([np.asarray(in_maps[c][name]) for c in range(N_CORES)], axis=0)
            for name in in_names
        ]
        concat_zeros = [np.zeros((N_CORES * z.shape[0], *z.shape[1:]), z.dtype)
                        for z in zero_outs]
        return concat_in, concat_zeros

    def run(in_maps, unpack=True):
        concat_in, concat_zeros = _concat(in_maps)
        out_arrs = sharded(*concat_in, *concat_zeros)
        if not unpack:
            jax.block_until_ready(out_arrs)
            return None
        return [
            {name: np.asarray(out_arrs[i]).reshape(N_CORES, *out_avals[i].shape)[c]
             for i, name in enumerate(out_names)}
            for c in range(N_CORES)
        ]

    def stage(in_maps):
        """device_put all operands once; returns args for timed_call."""
        from jax.sharding import NamedSharding
        sh = NamedSharding(mesh, PartitionSpec("core"))
        concat_in, concat_zeros = _concat(in_maps)
        dev = [jax.device_put(x, sh) for x in concat_in + concat_zeros]
        jax.block_until_ready(dev)
        return dev

    def timed_call(dev_args):
        out_arrs = sharded_nd(*dev_args)
        jax.block_until_ready(out_arrs)
        return out_arrs

    run.stage = stage
    run.timed_call = timed_call
    return run


def _get_runner():
    if "runner" not in _CACHE:
        nc = _build()
        _CACHE["nc"] = nc
        _CACHE["runner"] = _make_runner(nc)
    return _CACHE["runner"]


def _lrope_embT(label_emb, labels):
    inv_freq = (1.0 / (10000.0 ** (np.arange(0, HEAD_DIM, 2, dtype=np.float32)
                                   / HEAD_DIM))).astype(np.float32)
    pos = np.arange(SKV, dtype=np.float32)
    freqs = np.outer(pos, inv_freq)
    emb = np.concatenate([np.sin(freqs), np.cos(freqs)], axis=-1).astype(np.float32)
    lab = np.asarray(label_emb, np.float32)[np.asarray(labels).astype(np.int64)]
    return emb, lab  # [SKV, HD], [B, HD]


def make_in_maps(visual_features, audio_features, audio_labels,
                 Wq, bq, Wk, bk, Wv, bv, Wo, bo, label_emb):
    vis = np.asarray(visual_features, np.float32)
    aud = np.asarray(audio_features, np.float32)
    Wq = np.asarray(Wq, np.float32)
    Wk = np.asarray(Wk, np.float32)
    Wv = np.asarray(Wv, np.float32)
    Wo = np.asarray(Wo, np.float32)
    bq = np.asarray(bq, np.float32)
    bk = np.asarray(bk, np.float32)
    emb, lab = _lrope_embT(label_emb, audio_labels)

    xqs = [np.ascontiguousarray(vis[b].T).astype(np.float16) for b in range(B)]
    xas = [np.ascontiguousarray(aud[b].T).astype(np.float16) for b in range(B)]
    embs = []
    for b in range(B):
        embT = np.ascontiguousarray((emb * lab[b][None, :]).T) * QS  # [64, SKV]
        embs.append(np.concatenate([embT, embT], axis=0).astype(np.float16))

    in_maps = []
    for core in range(N_CORES):
        b, g = core // HPC, core % HPC
        sl = slice(g * DSL, (g + 1) * DSL)
        in_maps.append({
            "xq": xqs[b],
            "xa": xas[b],
            "wq": np.ascontiguousarray(Wq[sl, :].T * QS).astype(np.float16),
            "wk": np.ascontiguousarray(Wk[sl, :].T * QS).astype(np.float16),
            "wv": np.ascontiguousarray(Wv[sl, :].T).astype(np.float16),
            "wo": np.ascontiguousarray(Wo[:, sl].T).astype(np.float16),
            "emb2": embs[b],
            "bq2": np.ascontiguousarray(bq[sl].reshape(2, 128).T) * QS,
            "bk2": np.ascontiguousarray(bk[sl].reshape(2, 128).T) * QS,
        })
    return in_maps


def kernel(**inputs):
    run = _get_runner()
    in_maps = make_in_maps(**inputs)
    results = run(in_maps)
    bo = np.asarray(inputs["bo"], np.float32)
    bv = np.asarray(inputs["bv"], np.float32)
    Wo = np.asarray(inputs["Wo"], np.float32)
    bo_eff = bo + Wo @ bv
    out = np.empty((B, SQ, DIM), np.float32)
    for b in range(B):
        s = results[4 * b]["out"].astype(np.float32)
        for g in range(1, HPC):
            s = s + results[4 * b + g]["out"].astype(np.float32)
        out[b] = s.T + bo_eff[None, :]
    return out
